# revision 42
# baseline (speedup 1.0000x reference)
"""Trainium2 Bass kernel for nn_BaselineTrustModel.

Math (see the reference): the per-timestep recurrence is affine and collapses
to a per-sample scalar formula.  With
    s    = sum_t perf[t, n]                (number of "fail" flags, 0..T)
    mask = any(obs[0, n, :] != 0)
    r1   = 1/sqrt(sigma0^2 + T*sigma_t^2)
    z0   = trust0/sqrt(sigma0^2)
    A    = (trust0 + T*wb + T*wtp) * r1
    B    = 2*wtp*r1
the output is
    pred[n] = clip(sigmoid(z0 + mask*( (A - z0) - B*s )), 0.01, 0.99)

Only obs[0] (N x D) and perf (T x N) are ever read -> ~66 MB of f32 input
traffic total, data-parallel over the sample axis N across 8 cores
(~8.3 MB per core, memory-bound; per-core HBM roofline ~358 GB/s -> ~23 us).

Device kernel per core (raw bacc, hand-scheduled static schedule; no
TileContext so the kernel tail is one barrier + sem clear instead of Tile's
~10 us butterfly).  Partition p owns samples [p*F, (p+1)*F), F = 490.

  SP  : 16 perf t-layer loads [128 x 490] (HWDGE, DRAM-sequential),
        final store.  ~4.2 MB on this queue.
  ACT : 5 obs chunk loads [128 x 98*16] on its own HWDGE queue (~4.0 MB),
        then the sigmoid.
  Q7  : accumulates the 5 earliest perf layers (gpsimd elementwise adds),
        tail semaphore clear.
  DVE : accumulates the other 11 perf layers, 5 segmented abs-max obs
        reduces, epilogue d = s*(-B)+(A-z0); x = (ma>0)*d; clip.
"""

import math
import sys
from contextlib import ExitStack

import numpy as np

for _p in ("/opt/trn_rl_repo", "/root/.axon_site/_ro/trn_rl_repo"):
    if _p not in sys.path:
        sys.path.append(_p)

T = 16
D = 16
N = 500000
NCORES = 8

F = 490            # samples per partition per core
K = 5              # obs load/reduce chunks (F % K == 0)
NQ7 = 5            # perf layers accumulated by gpsimd (layers 0..NQ7-1)
NPB = 8            # perf layer buffers
PER = 128 * F      # 62720 samples per core
NPAD = NCORES * PER


def build_program(neg_b, c_const, z0):
    """Raw-bacc single-core program (SPMD across cores)."""
    from concourse import bacc, mybir

    f32 = mybir.dt.float32
    fc = F // K                      # 98 samples per obs chunk per partition
    nc = bacc.Bacc("TRN2", target_bir_lowering=False, debug=False)
    obs_d = nc.dram_tensor("obs0", [128, K, fc * D], f32, kind="ExternalInput").ap()
    perf_d = nc.dram_tensor("perfc", [T, 128, F], f32, kind="ExternalInput").ap()
    out_d = nc.dram_tensor("out", [128, F], f32, kind="ExternalOutput").ap()

    with ExitStack() as ctx:
        sb = lambda name, shape: ctx.enter_context(nc.sbuf_tensor(name, shape, f32))
        pb = [sb(f"pb{j}", [128, F]) for j in range(NPB)]       # perf layer bufs
        ob = [sb(f"ob{j}", [128, fc * D]) for j in range(3)]    # obs chunk bufs
        sA = sb("sA", [128, F])      # DVE partial sum (layers NQ7..15)
        sB = sb("sB", [128, F])      # Q7 partial sum (layers 0..NQ7-1)
        ss = sb("ss", [128, F])      # total s
        ma = sb("ma", [128, F])
        dd = sb("dd", [128, F])
        xx = sb("xx", [128, F])
        pp = sb("pp", [128, F])
        oo = sb("oo", [128, F])
        z0t = sb("z0t", [128, 1])

        pdma = [ctx.enter_context(nc.semaphore(f"pdma{j}")) for j in range(NPB)]
        obdma = [ctx.enter_context(nc.semaphore(f"obdma{j}")) for j in range(3)]
        odma = ctx.enter_context(nc.semaphore("odma"))
        dve = ctx.enter_context(nc.semaphore("dve"))
        q7 = ctx.enter_context(nc.semaphore("q7"))
        act = ctx.enter_context(nc.semaphore("act"))
        all_sems = pdma + obdma + [odma, dve, q7, act]
        nums = sorted(s.num for s in all_sems)
        assert nums == list(range(nums[0], nums[0] + len(nums))), nums
        sem_range = range(nums[0], nums[-1] + 1)

        # ---- static schedule bookkeeping ---------------------------------
        # Q7 ops (counter q7): a1=l0+l1, a2..a{NQ7-1}: sB += l_i
        #   -> layer i<NQ7 consumed at q7 >= max(1, i)
        # DVE ops (counter dve), in emission order:
        #   memset, then interleaved: b1=l5+l6, b_k: sA += l_i (i=7..15),
        #   r_k (obs reduces), then s=sA+sB, d, x, [wait act] clip
        # DVE add order interleaved with reduces to match arrival pacing:
        dve_order = ["b5", "b7", "r0", "b8", "b9", "r1", "b10", "b11",
                     "r2", "b12", "b13", "r3", "b14", "r4", "b15"]
        n = 1  # memset
        ldone, rdone = {}, {}   # layer-consumed / reduce-done dve counts
        for tok in dve_order:
            n += 1
            if tok[0] == "b":
                i = int(tok[1:])
                ldone[i] = n
                if i == 5:
                    ldone[6] = n  # b5 is l5+l6
            else:
                rdone[int(tok[1])] = n
        s_n, d_n, x_n, clip_n = n + 1, n + 2, n + 3, n + 4

        def layer_war_wait(eng, i):
            """Before loading layer i into slot i%NPB, wait until layer
            i-NPB was consumed."""
            j = i - NPB
            if j < 0:
                return
            if j < NQ7:
                eng.wait_ge(q7, max(1, j))
            else:
                eng.wait_ge(dve, ldone[j])

        block_cm = nc.Block()
        block = block_cm.__enter__()

        @block.sync
        def _(sync):
            for i in range(T):
                layer_war_wait(sync, i)
                sync.dma_start(pb[i % NPB][:], perf_d[i]).then_inc(
                    pdma[i % NPB], 16
                )
            sync.wait_ge(dve, clip_n)
            sync.dma_start(out_d[:], oo[:]).then_inc(odma, 16)
            sync.wait_ge(odma, 16)

        @block.scalar
        def _(scalar):
            for k in range(K):
                if k >= 3:
                    scalar.wait_ge(dve, rdone[k - 3])
                scalar.dma_start(ob[k % 3][:], obs_d[:, k]).then_inc(obdma[k % 3], 16)
            scalar.wait_ge(dve, x_n)
            nc.scalar.activation(
                pp[:], xx[:], mybir.ActivationFunctionType.Sigmoid,
                bias=z0t[:], scale=1.0,
            ).then_inc(act, 1)

        @block.gpsimd
        def _(gpsimd):
            # accumulate layers 0..NQ7-1 into sB (slots 0..NQ7-1, first use)
            gpsimd.wait_ge(pdma[1], 16)
            gpsimd.wait_ge(pdma[0], 16)
            nc.gpsimd.tensor_add(sB[:], pb[0][:], pb[1][:]).then_inc(q7, 1)
            for i in range(2, NQ7):
                gpsimd.wait_ge(pdma[i], 16)
                gpsimd.wait_ge(q7, i - 1)  # RAW sB (pipelined write-back)
                nc.gpsimd.tensor_add(sB[:], sB[:], pb[i][:]).then_inc(q7, 1)

        @block.vector
        def _(vector):
            cnt = [0]

            def emit(instr):
                instr.then_inc(dve, 1)
                cnt[0] += 1
                return cnt[0]

            emit(nc.vector.memset(z0t[:], z0))
            have_sA = False

            def badd(i):
                nonlocal have_sA
                slot = i % NPB
                vector.wait_ge(pdma[slot], 16 * (i // NPB + 1))
                if not have_sA:
                    # b5 = l5 + l6 needs both slots
                    slot6 = 6 % NPB
                    vector.wait_ge(pdma[slot6], 16)
                    emit(nc.vector.tensor_add(sA[:], pb[slot][:], pb[slot6][:]))
                    have_sA = True
                else:
                    vector.wait_ge(dve, cnt[0])  # RAW sA write-back
                    emit(nc.vector.tensor_add(sA[:], sA[:], pb[slot][:]))

            def reduce(k):
                vector.wait_ge(obdma[k % 3], 16 * (k // 3 + 1))
                emit(nc.vector.tensor_reduce(
                    ma[:, k * fc:(k + 1) * fc],
                    ob[k % 3][:].rearrange("p (f d) -> p f d", d=D),
                    axis=mybir.AxisListType.X,
                    op=mybir.AluOpType.max,
                    apply_absolute_value=True,
                ))

            for tok in dve_order:
                if tok[0] == "b":
                    badd(int(tok[1:]))
                else:
                    reduce(int(tok[1]))

            vector.wait_ge(q7, NQ7 - 1)
            vector.wait_ge(dve, cnt[0])
            emit(nc.vector.tensor_add(ss[:], sA[:], sB[:]))
            vector.wait_ge(dve, cnt[0])
            emit(nc.vector.tensor_scalar(
                dd[:], ss[:], neg_b, c_const,
                op0=mybir.AluOpType.mult, op1=mybir.AluOpType.add,
            ))
            vector.wait_ge(dve, cnt[0])
            emit(nc.vector.scalar_tensor_tensor(
                xx[:], ma[:], 0.0, dd[:],
                op0=mybir.AluOpType.is_gt, op1=mybir.AluOpType.mult,
            ))
            vector.wait_ge(act, 1)
            emit(nc.vector.tensor_scalar(
                oo[:], pp[:], 0.01, 0.99,
                op0=mybir.AluOpType.max, op1=mybir.AluOpType.min,
            ))
            assert cnt[0] == clip_n, (cnt[0], clip_n)

        block_cm.__exit__(None, None, None)
        # Re-executable NEFF tail (the NTFF profiler replays it): one
        # all-engine barrier, then zero our semaphores.
        nc.all_engine_barrier()
        nc.gpsimd.dma_reset(sem_range)
        nc.gpsimd.sem_clear(sem_range)

    nc.compile()
    return nc


def _scalar_constants(inputs):
    t0 = float(np.asarray(inputs["trust0"]).reshape(()))
    s0 = float(np.asarray(inputs["sigma0"]).reshape(()))
    wb = float(np.asarray(inputs["wb"]).reshape(()))
    wtp = float(np.asarray(inputs["wtp"]).reshape(()))
    st = float(np.asarray(inputs["sigma_t"]).reshape(()))
    r1 = 1.0 / math.sqrt(s0 * s0 + T * st * st)
    z0 = t0 / math.sqrt(s0 * s0)
    a_const = (t0 + T * wb + T * wtp) * r1
    neg_b = -2.0 * wtp * r1
    c_const = a_const - z0
    return neg_b, c_const, z0


def run(inputs, trace=False, **kw):
    """Shard, run on 8 cores, gather. Returns (output [N,1] f32, exec_time_ns)."""
    from concourse.bass_utils import run_bass_kernel_spmd

    obs = np.asarray(inputs["inptasksobs"])
    perf = np.asarray(inputs["inptasksperf"])
    assert obs.shape == (T, N, D) and perf.shape == (T, N, 1)

    neg_b, c_const, z0 = _scalar_constants(inputs)
    nc = build_program(neg_b, c_const, z0)

    obs_p = np.zeros((NPAD, D), np.float32)
    obs_p[:N] = obs[0]
    perf_p = np.zeros((T, NPAD), np.float32)
    perf_p[:, :N] = perf[:, :, 0]

    in_maps = []
    for c in range(NCORES):
        oc = obs_p[c * PER:(c + 1) * PER].reshape(128, K, (F // K) * D)
        pc = np.ascontiguousarray(
            perf_p[:, c * PER:(c + 1) * PER]
        ).reshape(T, 128, F)
        in_maps.append({"obs0": oc, "perfc": pc})

    res = run_bass_kernel_spmd(
        nc, in_maps, core_ids=list(range(NCORES)), trace=trace, **kw
    )
    full = np.concatenate(
        [res.results[c]["out"].reshape(-1) for c in range(NCORES)]
    )
    return full[:N].reshape(N, 1).astype(np.float32, copy=False), res.exec_time_ns


def kernel(**inputs):
    out, _ = run(inputs, trace=False)
    return out
